# revision 2
# baseline (speedup 1.0000x reference)
"""GCNConv (D^-1/2 A D^-1/2 X W + b) on 8 Trainium2 NeuronCores.

Row-sharded over nodes: each core owns a [1024, 8192] row block of the
adjacency and the matching rows of input_feature; weight/bias replicated.

Pipeline per core:
  - support = X @ W (bf16) for the local rows, kept in SBUF.
  - the adjacency block streams ONCE from HBM in [128, 2048] f32 chunks;
    each chunk is cast f32->bf16 (ACT/DVE alternating, with the rowsum
    accumulated for free via accum_out) and transposed on the TensorEngine
    (identity matmul, 4x 128x128 per PSUM bank) into a resident bf16 A^T.
  - d = (rowsum + l)^-1/2 per 128-row m-tile (sqrt + reciprocal + one
    Newton step) as soon as that m-tile finishes streaming.
  - the d-scaling is folded into the support AllGather: each core scales
    its OWN support rows by its OWN d and AllGathers the already-scaled
    SV in `dg`=4 chunks of 2 phases each (so only 4 cheap collectives and
    no consumer-side scaling); chunk loads are batched contiguous DMAs.
  - main matmul (out[m,n] += A^T[k,m].T @ SV[k,n]) runs after the stream:
    phases 0-5 first (~40us of AG-independent PE work that covers the
    last AG chunk's completion latency), then phases 6-7 m-tile-major so
    each m-tile's A^T slots and output release early -- this lets the
    next repetition's stream pipeline behind the matmul block instead of
    serializing on tile reuse.
  - epilogue per m-tile: scale rows by d_m, add broadcast bias, store.
"""
import sys
sys.path.insert(0, "/opt/trn_rl_repo")
from contextlib import ExitStack

import numpy as np

import concourse.bass as bass
import concourse.bacc as bacc
import concourse.tile as tile
import concourse.bass_utils as bass_utils
import concourse.mybir as mybir

N_CORES = 8
N = 8192
DIN = 256
DOUT = 256
P = 128
M_LOC = N // N_CORES          # 1024 rows per core
MT = M_LOC // P               # 8 m-tiles per core (= k-phases per core)
KT = N // P                   # 64 k-tiles global
GT = KT // 4                  # 16 k-quad groups
CHUNK = 2048                  # k-chunk per streaming DMA
NCH = N // CHUNK              # 4 chunks per m-tile
NQ = CHUNK // (4 * P)         # 4 k-quads per chunk
F32 = mybir.dt.float32
BF16 = mybir.dt.bfloat16
RG = [list(range(N_CORES))]
Alu = mybir.AluOpType
ActF = mybir.ActivationFunctionType
AxX = mybir.AxisListType.X
WAVE_BUDGET = 0               # max (phase, m-tile) pairs issued per m-tile
STREAM_DUAL = False           # alternate natf DMAs between sync and gpsimd rings
CHUNK_MAP = {                 # dg -> scaled-support AG chunks (p_lo, p_hi)
    2: [(0, 4), (4, 8)],
    4: [(0, 2), (2, 4), (4, 6), (6, 8)],
    8: [(p, p + 1) for p in range(8)],
    21: [(0, 7), (7, 8)],
    31: [(0, 4), (4, 7), (7, 8)],
}


def _emit_body(nc, tc, pools, consts, rep, stage="full", dg=4):
    do_coll = stage in ("coll", "sv", "full")
    do_sv = stage in ("sv", "full")
    do_mm = stage == "full"
    (natp, natbp, supp, xtp, atpp, svp, dtp, stagep, tpp, mmp,
     dram) = pools
    (ident, wb, bias_bc, lv, a, x, w, bias, out) = consts
    R = f"r{rep}_"
    chunks = CHUNK_MAP[dg]        # list of (p_lo, p_hi) AG chunks
    ndg = len(chunks)
    chunk_of = {}
    for cc, (lo, hi) in enumerate(chunks):
        for p in range(lo, hi):
            chunk_of[p] = cc

    # ---- DRAM bounce buffers for the scaled-support AllGather chunks ----
    sag_in = [dram.tile([(hi - lo) * P, DOUT], BF16, tag=f"sag_in{c}",
                        name=R + f"sag_in{c}")
              for c, (lo, hi) in enumerate(chunks)]
    sag_out = [dram.tile([N_CORES * (hi - lo) * P, DOUT], BF16,
                         addr_space="Shared", tag=f"sag_out{c}",
                         name=R + f"sag_out{c}")
               for c, (lo, hi) in enumerate(chunks)]

    # ---- support = X @ W (bf16) per local m-tile, kept in SBUF ----
    xt = [xtp.tile([P, M_LOC], BF16, tag="xt", name=R + f"xt{dt}")
          for dt in range(DIN // P)]
    for i in range(MT):
        xb = supp.tile([P, DIN], BF16, tag="xb", bufs=2, name=R + f"xb{i}")
        nc.gpsimd.dma_start(xb[:], x.ap()[i * P:(i + 1) * P, :])
        for dt in range(DIN // P):
            ps = tpp.tile([P, 512], F32, tag="tp", name=R + f"xps{i}_{dt}")
            nc.tensor.matmul(ps[:, 0:P], xb[:, dt * P:(dt + 1) * P], ident[:],
                             start=True, stop=True)
            nc.vector.tensor_copy(xt[dt][:, i * P:(i + 1) * P], ps[:, 0:P])
    sst = []
    for i in range(MT):
        sps_t = tpp.tile([P, 512], F32, tag="tp", name=R + f"sps{i}")
        sps = sps_t[:, 0:DOUT]
        for dt in range(DIN // P):
            nc.tensor.matmul(sps, xt[dt][:, i * P:(i + 1) * P], wb[dt][:],
                             start=(dt == 0), stop=(dt == DIN // P - 1))
        st = supp.tile([P, DOUT], BF16, tag="sst", bufs=MT,
                       name=R + f"sst{i}")
        nc.scalar.copy(st[:], sps)
        sst.append(st)

    # ---- resident transposed adjacency (bf16) and SV blocks ----
    atp = {}
    for g in range(GT):
        for i in range(MT):
            atp[(g, i)] = atpp.tile([P, 512], BF16, tag="atp",
                                    name=R + f"atp_{g}_{i}")
    svblk = [None] * ndg

    par = dtp.tile([P, MT * NCH], F32, tag="par", name=R + "par")
    dcols = dtp.tile([P, MT], F32, tag="dcols", name=R + "dcols")

    mmps = [mmp.tile([P, 512], F32, tag="mmps", name=R + f"mmps_{b}")
            for b in range(MT // 2)]
    mm_started = [False] * (MT // 2)

    def sv_load(cc):
        # batched loads of the AG'd scaled-support chunk into one block
        # tile; one DMA per 64KB contiguous [128, 256] slice
        lo, hi = chunks[cc]
        nq = N_CORES * (hi - lo)
        svblk[cc] = svp.tile([P, nq * DOUT], BF16, tag=f"svblk{cc}",
                             bufs=1, name=R + f"svblk{cc}")
        for q in range(nq):
            nc.gpsimd.dma_start(
                svblk[cc][:, q * DOUT:(q + 1) * DOUT],
                sag_out[cc][q * P:(q + 1) * P, :])

    def sv_ap(t):
        r, p = t // MT, t % MT
        cc = chunk_of[p]
        lo, hi = chunks[cc]
        q = r * (hi - lo) + (p - lo)
        return svblk[cc][:, q * DOUT:(q + 1) * DOUT]

    def mm_pairs(pairs, final=False):
        for (p, i) in pairs:
            b = i // 2
            dst = mmps[b][:, (i % 2) * DOUT:(i % 2 + 1) * DOUT]
            for r in range(N_CORES):
                t = r * MT + p
                g = 2 * r + p // 4
                st = (not mm_started[b]) and i % 2 == 0
                mm_started[b] = True
                nc.tensor.matmul(
                    dst, atp[(g, i)][:, (p % 4) * P:(p % 4 + 1) * P],
                    sv_ap(t), start=st,
                    stop=final and p == MT - 1 and r == N_CORES - 1,
                    skip_group_check=True)

    def epilogue(i):
        # scale rows by d_m, add bias, store
        src = mmps[i // 2][:, (i % 2) * DOUT:(i % 2 + 1) * DOUT]
        st1 = stagep.tile([P, DOUT], F32, tag="stage", name=R + f"st1_{i}")
        nc.vector.tensor_scalar_mul(st1[:], src, dcols[:, i:i + 1])
        st2 = stagep.tile([P, DOUT], F32, tag="stage", name=R + f"st2_{i}")
        nc.vector.tensor_add(st2[:], st1[:], bias_bc[:])
        nc.sync.dma_start(out.ap()[i * P:(i + 1) * P, :], st2[:])

    emitted = set()
    loaded_phases = set()
    fired = []                     # chunk cc -> tau it was fired at

    # ---- main stream over the adjacency block ----
    for tau in range(MT):
        for j in range(NCH):
            natf = natp.tile([P, CHUNK], F32, tag="nat",
                             name=R + f"natf{tau}_{j}")
            dma_eng = nc.gpsimd if (STREAM_DUAL and j % 2 == 1) else nc.sync
            dma_eng.dma_start(
                natf[:],
                a.ap()[tau * P:(tau + 1) * P, j * CHUNK:(j + 1) * CHUNK])
            nat = natbp.tile([P, CHUNK], BF16, tag="natb",
                             name=R + f"nat{tau}_{j}")
            c = tau * NCH + j
            if j % 2 == 0:
                nc.scalar.activation(nat[:], natf[:], ActF.Copy,
                                     accum_out=par[:, c:c + 1])
            else:
                nc.vector.tensor_scalar(nat[:], natf[:], 1.0, None,
                                        op0=Alu.mult, op1=Alu.add,
                                        accum_out=par[:, c:c + 1])
            for q in range(NQ):
                g = j * NQ + q
                ps = tpp.tile([P, 512], F32, tag="tp",
                              name=R + f"tps{tau}_{g}")
                for u in range(4):
                    s = q * 4 + u
                    nc.tensor.matmul(ps[:, u * P:(u + 1) * P],
                                     nat[:, s * P:(s + 1) * P], ident[:],
                                     start=True, stop=True)
                if g % 2 == 0:
                    nc.vector.tensor_copy(atp[(g, tau)][:], ps[:])
                else:
                    nc.scalar.copy(atp[(g, tau)][:], ps[:])
        # degree -> d for this m-tile
        deg = dtp.tile([P, 1], F32, tag="deg", bufs=2, name=R + f"deg{tau}")
        nc.vector.tensor_reduce(deg[:], par[:, tau * NCH:(tau + 1) * NCH],
                                axis=AxX, op=Alu.add)
        deg2 = dtp.tile([P, 1], F32, tag="deg2", bufs=2, name=R + f"deg2{tau}")
        nc.vector.tensor_scalar_add(deg2[:], deg[:], lv[:])
        s0 = dtp.tile([P, 1], F32, tag="s0", bufs=2, name=R + f"s0{tau}")
        nc.scalar.sqrt(s0[:], deg2[:])
        r0 = dtp.tile([P, 1], F32, tag="r0", bufs=2, name=R + f"r0{tau}")
        nc.vector.reciprocal(r0[:], s0[:])
        # one Newton step: d = r0 * (1.5 - 0.5 * deg2 * r0^2)
        t1 = dtp.tile([P, 1], F32, tag="t1", bufs=2, name=R + f"t1{tau}")
        nc.vector.tensor_mul(t1[:], r0[:], r0[:])
        t2 = dtp.tile([P, 1], F32, tag="t2", bufs=2, name=R + f"t2{tau}")
        nc.vector.tensor_mul(t2[:], t1[:], deg2[:])
        t3 = dtp.tile([P, 1], F32, tag="t3", bufs=2, name=R + f"t3{tau}")
        nc.vector.tensor_scalar(t3[:], t2[:], -0.5, 1.5, op0=Alu.mult,
                                op1=Alu.add)
        nc.vector.tensor_mul(dcols[:, tau:tau + 1], r0[:], t3[:])
        # fire the scaled-support AG once the chunk's last phase d is ready
        if do_coll and tau in {hi - 1 for (lo, hi) in chunks}:
            cc = [c for c, (lo, hi) in enumerate(chunks) if hi - 1 == tau][0]
            lo, hi = chunks[cc]
            for p in range(lo, hi):
                svt = supp.tile([P, DOUT], BF16, tag="svt", bufs=4,
                                name=R + f"svt{p}")
                nc.vector.tensor_scalar_mul(svt[:], sst[p][:],
                                            dcols[:, p:p + 1])
                nc.scalar.dma_start(sag_in[cc][(p - lo) * P:(p - lo + 1) * P,
                                               :], svt[:])
            nc.gpsimd.collective_compute(
                "AllGather", Alu.bypass, replica_groups=RG,
                ins=[sag_in[cc].opt()], outs=[sag_out[cc].opt()])
            fired.append((cc, tau))
        # consume: load sv chunks AG'd >= 1 m-tile ago, emit matmul waves
        if do_sv:
            for (cc, ft) in fired:
                if ft <= tau - 1 and cc not in loaded_phases:
                    loaded_phases.add(cc)
                    sv_load(cc)
        if do_mm:
            avail = [p for cc in loaded_phases
                     for p in range(chunks[cc][0], chunks[cc][1])]
            pairs = [(p, i) for p in sorted(avail) for i in range(tau + 1)
                     if (p, i) not in emitted]
            pairs = pairs[:WAVE_BUDGET]   # cap PE work per m-tile slot
            emitted.update(pairs)
            mm_pairs(pairs)

    # ---- tail: remaining chunks, then epilogue ----
    if do_sv:
        for cc in range(ndg):
            if cc not in loaded_phases:
                loaded_phases.add(cc)
                sv_load(cc)
    if not do_mm:
        return
    # phases 0..MT-3 first (p-major): ~40us of AG-independent PE work that
    # covers the last AG chunk's completion latency; then the last two
    # phases i-major so each m-tile's transposed-A slots (and its output)
    # release as early as possible for the next repetition's stream.
    pairs = [(p, i) for p in range(MT - 2) for i in range(MT)
             if (p, i) not in emitted]
    mm_pairs(pairs, final=True)
    for i in range(MT):
        pairs = [(p, i) for p in (MT - 2, MT - 1) if (p, i) not in emitted]
        mm_pairs(pairs, final=True)
        epilogue(i)


def build(repeat=1, stage="full", dg=4):
    nc = bacc.Bacc("TRN2", target_bir_lowering=False, debug=False,
                   num_devices=N_CORES)
    a = nc.dram_tensor("a", [M_LOC, N], F32, kind="ExternalInput")
    x = nc.dram_tensor("x", [M_LOC, DIN], F32, kind="ExternalInput")
    w = nc.dram_tensor("w", [DIN, DOUT], F32, kind="ExternalInput")
    bias = nc.dram_tensor("bias", [DOUT], F32, kind="ExternalInput")
    lvec = nc.dram_tensor("lvec", [P, 1], F32, kind="ExternalInput")
    out = nc.dram_tensor("out", [M_LOC, DOUT], F32, kind="ExternalOutput")

    with tile.TileContext(nc) as tc, ExitStack() as ctx:
        cpool = ctx.enter_context(tc.tile_pool(name="cpool", bufs=1))
        natp = ctx.enter_context(tc.tile_pool(name="natp", bufs=2))
        natbp = ctx.enter_context(tc.tile_pool(name="natbp", bufs=3))
        supp = ctx.enter_context(tc.tile_pool(name="supp", bufs=4))
        xtp = ctx.enter_context(tc.tile_pool(name="xtp", bufs=2))
        atpp = ctx.enter_context(tc.tile_pool(name="atpp", bufs=GT * MT))
        svp = ctx.enter_context(tc.tile_pool(name="svp", bufs=2))
        dtp = ctx.enter_context(tc.tile_pool(name="dtp", bufs=1))
        stagep = ctx.enter_context(tc.tile_pool(name="stagep", bufs=2))
        tpp = ctx.enter_context(tc.tile_pool(name="tpp", bufs=4, space="PSUM"))
        mmp = ctx.enter_context(tc.tile_pool(name="mmp", bufs=MT // 2,
                                             space="PSUM"))
        dram = ctx.enter_context(tc.tile_pool(name="dram", bufs=1,
                                              space="DRAM"))

        # ---- constants ----
        ones_bf = cpool.tile([P, P], BF16)
        nc.vector.memset(ones_bf[:], 1.0)
        ident = cpool.tile([P, P], BF16)
        nc.gpsimd.affine_select(
            ident[:], ones_bf[:], pattern=[[1, P]],
            compare_op=Alu.is_equal, fill=0.0, base=0, channel_multiplier=-1)
        wb = []
        for dt in range(DIN // P):
            wt = cpool.tile([P, DOUT], BF16, tag=f"wb{dt}", name=f"wb{dt}")
            nc.gpsimd.dma_start(wt[:], w.ap()[dt * P:(dt + 1) * P, :])
            wb.append(wt)
        lv = cpool.tile([P, 1], F32, tag="lv")
        nc.scalar.dma_start(lv[:], lvec.ap())
        # broadcast bias over partitions with a K=1 matmul
        ones_row = cpool.tile([1, P], F32, tag="ones_row")
        nc.vector.memset(ones_row[:], 1.0)
        bias_row = cpool.tile([1, DOUT], F32, tag="bias_row")
        nc.scalar.dma_start(bias_row[:], bias.ap()[None, :])
        bias_bc = cpool.tile([P, DOUT], F32, tag="bias_bc")
        bps = tpp.tile([P, 512], F32, tag="tp", name="bias_ps")
        nc.tensor.matmul(bps[:, 0:DOUT], ones_row[:], bias_row[:],
                         start=True, stop=True)
        nc.vector.tensor_copy(bias_bc[:], bps[:, 0:DOUT])

        pools = (natp, natbp, supp, xtp, atpp, svp, dtp, stagep,
                 tpp, mmp, dram)
        consts = (ident, wb, bias_bc, lv, a, x, w, bias, out)
        for rep in range(repeat):
            _emit_body(nc, tc, pools, consts, rep, stage=stage, dg=dg)
    nc.compile()
    return nc


def make_in_maps(adjacency, input_feature, weight, bias, l):
    adjacency = np.ascontiguousarray(np.asarray(adjacency, dtype=np.float32))
    input_feature = np.ascontiguousarray(
        np.asarray(input_feature, dtype=np.float32))
    weight = np.ascontiguousarray(np.asarray(weight, dtype=np.float32))
    bias_np = np.ascontiguousarray(np.asarray(bias, dtype=np.float32))
    lval = float(np.asarray(l))
    lv = np.full((P, 1), lval, dtype=np.float32)
    in_maps = []
    for c in range(N_CORES):
        in_maps.append({
            "a": adjacency[c * M_LOC:(c + 1) * M_LOC, :],
            "x": input_feature[c * M_LOC:(c + 1) * M_LOC, :],
            "w": weight,
            "bias": bias_np,
            "lvec": lv,
        })
    return in_maps


_NC_CACHE = None


def kernel(adjacency, input_feature, weight, bias, l):
    global _NC_CACHE
    if _NC_CACHE is None:
        _NC_CACHE = build()
    nc = _NC_CACHE
    in_maps = make_in_maps(adjacency, input_feature, weight, bias, l)
    res = None
    last_err = None
    for attempt in range(3):
        try:
            res = bass_utils.run_bass_kernel_spmd(
                nc, in_maps, core_ids=list(range(N_CORES)))
            break
        except Exception as e:           # transient device wedge: retry
            last_err = e
            import time as _time
            _time.sleep(5.0 * (attempt + 1))
    if res is None:
        raise last_err
    blocks = [res.results[c]["out"] for c in range(N_CORES)]
    return np.ascontiguousarray(np.concatenate(blocks, axis=0),
                                dtype=np.float32)


if __name__ == "__main__":
    rng = np.random.default_rng(0)
    A = rng.random((N, N), dtype=np.float32)
    X = rng.standard_normal((N, DIN)).astype(np.float32)
    W = (rng.standard_normal((DIN, DOUT)) / np.sqrt(DIN)).astype(np.float32)
    B = np.zeros((DOUT,), dtype=np.float32)
    out = kernel(A, X, W, B, 1)
    deg = A.sum(axis=1) + 1.0
    d = np.where(deg > 0, deg ** -0.5, 0.0).astype(np.float32)
    ref = (A * d[:, None] * d[None, :]) @ (X @ W) + B
    err = np.abs(out - ref)
    rel = np.linalg.norm(out - ref) / np.linalg.norm(ref)
    print(f"max abs err {err.max():.3e}  rel l2 {rel:.3e}")
